# revision 25
# baseline (speedup 1.0000x reference)
"""ClassBalancedSupConLoss on 8 TRN2 NeuronCores (Bass/Tile), v2.

Math (reference semantics, reorganized for hardware):
  - All embeddings are unit-norm; fixed logsumexp shift m = 1:
        LSE_i = inv_t_i + log( sum_j exp(inv_t_i * (s_ij - 1)) )
    Self term excluded by subtracting exp of the bitwise-identical
    on-device s_ii product.  Batch and bank sorted by class on host so
    same-class columns are contiguous segments.
  - Anchors sharded 256/core across 8 cores; full embT/bankT replicas
    per core.  Device outputs per-anchor (den, lin); host does the
    2048 logs + masked mean.

v2 changes vs baseline (65.6us):
  - Input DMAs issued from sync+gpsimd queues only -- the scalar (ACT)
    engine previously spent ~9.5us issuing DMA descriptors before its
    first exp.
  - The exp stream is SPLIT between the ACT engine (hardware exp LUT,
    1 elem/lane/cyc @ 1.2GHz) and the Vector engine via two custom DVE
    ops: exp(z) ~= p3(z/128)^128 where p3 is a minimax cubic in
    factored form (pass A, 6 ALU slices) and pass B is 7 inline
    squarings with a free row-sum accumulator.  Max rel err 3.7e-4.
  - Anchor (stationary) operands are pre-scaled by inv_t/128 on host,
    so PSUM holds z/128 directly: ACT chunks use exp(128*x - inv_t)
    (free affine), DVE pass A needs only a per-partition shift.

SPMD: one program for all 8 cores; per-core data in the packed `vecs`
tile.
"""

import os
import numpy as np

import concourse.bass as bass  # noqa: F401
from concourse import bacc
import concourse.mybir as mybir
import concourse.tile as tile
from concourse.bass_utils import run_bass_kernel_spmd

B, D, M, C = 2048, 128, 16384, 3
NCORES = 8
APC = B // NCORES          # anchors per core = 256
NT = APC // 128            # anchor tiles per core = 2
CH = 512                   # matmul free chunk (one PSUM bank)
W = 2048                   # big PSUM chunk (4 banks) = one exp pass
NBK = M // W               # 8 bank pieces of [128, 2048]
BASE_TEMP = 0.07

F32 = mybir.dt.float32
BF16 = mybir.dt.bfloat16
AF = mybir.ActivationFunctionType
ALU = mybir.AluOpType
AX = mybir.AxisListType

LAST_EXEC_TIME_NS = None   # set by kernel() when SUPCON_TRACE=1

# ---- custom DVE exp: exp(z) = p3(v)^128, v = z/128 - u0 --------------------
# p3 fit of e^u on u in [-0.26, 0] (z in [-33, 0]; terms below e^-33 are
# ~1e-14 of the row sum).  Factored: p = (v*k) * ((v + bp)*v + gp),
# v = Src0 + C0 with C0 = -inv_t/128 - u0 per-anchor.  Pass B: w^128 via
# 7 squarings, row-sum accumulated in-instruction (no READ_ACCUMULATOR).
EXP_U0 = -1.7295465562795673
EXP_K = 0.146172629836262
EXP_BP = -1.791396476586659
EXP_GP = 4.062464246444453
# k absorbed into the variable: w = cbrt(k)*v, p = w*((w+BPk)*w+GPk);
# host scales anchors by inv_t/128*cbrt(k), ACT uses scale 128/cbrt(k)
KCBRT = float(EXP_K ** (1.0 / 3.0))
EXP_BPK = float(EXP_BP * KCBRT)
EXP_GPK = float(EXP_GP * KCBRT * KCBRT)
ACT_SCALE = float(128.0 / KCBRT)

_EXP_OPS = {}


def _register_exp_ops():
    """Define + register the two custom DVE ops with concourse's tables.

    dve_table_for_ops resolves op names through dve_ops.OPS /
    _SUB_OPCODE_FOR_NAME / CUSTOM_DVE_SPECS, all module-level registries;
    new ops just take the next free 5-bit opcode rows (18 in use < 32).
    """
    global _EXP_OPS
    if _EXP_OPS:
        return _EXP_OPS
    from operator import add as _add
    import concourse.dve_ops as dom
    from concourse.dve_spec import Spec, Src0, Src1, C0, C1, C2, lower, sq
    from concourse.dve_spec import _has_src1
    from concourse.dve_uop import DveOpSpec

    # pass A: q = v*((v+bp)*v+gp), v = Src0 + C0; the leading k is applied
    # in pass B's first slice (q*C1 before the squarings) -- keeps pass A
    # at three constant slots with no Src1 stream.
    tmode = os.environ.get("SUPCON_DVE_SPEC", "fan3")
    if tmode == "fan2":
        va = Src0 + C0
        vb = Src0 + C0
        body_a = ((va + C1) * va + C2) * vb
    else:
        v = Src0 + C0
        body_a = v * ((v + C1) * v + C2)

    def ref_a(in0, in1, s0, s1, imm2):
        vv = in0.astype(np.float32) + np.asarray(s0, np.float32)
        return (vv * ((vv + s1) * vv + imm2)).astype(np.float32)

    body_b = sq(sq(sq(sq(sq(sq(sq(Src0)))))))

    def ref_b(in0, in1, s0, s1, imm2):
        b = in0.astype(np.float32)
        for _ in range(7):
            b = (b * b).astype(np.float32)
        acc = np.asarray(s0, np.float32).reshape(-1, 1) + b.reshape(
            b.shape[0], -1).sum(axis=-1, keepdims=True)
        return b, acc.astype(np.float32)

    spec_a = Spec(body=body_a, reference=ref_a)
    spec_b = Spec(body=body_b, accum=_add, accum_init=C0, reference=ref_b)

    # The DVE NX firmware dispatch table only knows the stock opcode rows,
    # so new rows would hang the engine.  Instead REPLACE the table
    # programs of two stock ops this kernel never calls -- the per-NEFF
    # uop table (qDveTable) is regenerated from dve_ops.OPS at compile
    # time, so the hijacked rows carry the exp programs.
    hijack = {"EXP_POLY_A_ANT": "CODY_WAITE_CASCADE",
              "EXP_SQ7_RED_ANT": "ADD_RANGE_WRAP"}
    for myname, spec in [("EXP_POLY_A_ANT", spec_a), ("EXP_SQ7_RED_ANT", spec_b)]:
        name = hijack[myname]
        idx = next(i for i, o in enumerate(dom.OPS) if o.name == name)
        if dom.OPS[idx].spec is spec:
            _EXP_OPS[myname] = dom.OPS[idx]
            continue
        row = dom._SUB_OPCODE_FOR_NAME[name]
        shas = {}
        for ver in ("v3", "v4"):
            try:
                r = DveOpSpec(name=name, opcode=row, uops=lower(spec, ver=ver),
                              rd1_en=_has_src1(spec))
                shas[ver] = r.sha(ver)
            except Exception:
                pass
        op = dom.DveOp(name, spec, subdim=False, uops_sha=shas)
        dom.OPS[idx] = op
        dom.CUSTOM_DVE_SPECS[name] = spec
        _EXP_OPS[myname] = op
    return _EXP_OPS


def _install_trace_shim():
    """Register the NTFF profile hook that this image's antenv lacks."""
    import sys
    import types
    import ctypes
    import contextlib

    try:
        from antenv.axon_hooks import get_axon_ntff_profile_hook  # noqa: F401
        return True
    except ImportError:
        pass

    so_path = "/opt/axon/libaxon_pjrt.so"
    if not os.path.exists(so_path):
        return False
    lib = ctypes.CDLL(so_path)
    if not hasattr(lib, "axon_start_nrt_profile"):
        return False
    lib.axon_start_nrt_profile.argtypes = [
        ctypes.POINTER(ctypes.c_int64),
        ctypes.c_size_t,
    ]
    lib.axon_start_nrt_profile.restype = ctypes.c_int64
    lib.axon_stop_nrt_profile.argtypes = [ctypes.c_char_p]
    lib.axon_stop_nrt_profile.restype = ctypes.c_int64

    @contextlib.contextmanager
    def _hook(output_dir, device_ids):
        import jax

        jax.devices()
        if device_ids:
            ids = (ctypes.c_int64 * len(device_ids))(*device_ids)
            rc = lib.axon_start_nrt_profile(ids, len(device_ids))
        else:
            rc = lib.axon_start_nrt_profile(None, 0)
        if rc != 0:
            raise RuntimeError(f"axon_start_nrt_profile rc={rc}")
        try:
            yield
        finally:
            n = lib.axon_stop_nrt_profile(str(output_dir).encode())
            print(f"profile: {n} file(s) written to {output_dir}", file=sys.stderr)

    _state = {"hook": _hook}
    mod = types.ModuleType("antenv.axon_hooks")
    mod.get_axon_ntff_profile_hook = lambda: _state["hook"]
    mod.set_axon_ntff_profile_hook = lambda h: _state.update(hook=h)
    sys.modules["antenv.axon_hooks"] = mod
    import antenv

    antenv.axon_hooks = mod

    import concourse.bass_utils as bu

    bu.upload_artifacts = lambda tmpdir: tmpdir
    return True


def _bank_subranges(mk_b1, mk_b2):
    """Split [0, M) at big-chunk multiples AND class boundaries."""
    cuts = sorted({c * W for c in range(NBK + 1)} | {mk_b1, mk_b2})
    subs = [(cuts[i], cuts[i + 1]) for i in range(len(cuts) - 1)]
    return subs


def _dve_chunks(mk_b1, mk_b2):
    """Per-tile bank chunk sets handled by the Vector engine (rest ACT).

    Boundary-containing chunks go to DVE (its per-call overhead is tiny,
    ACT pays ~600ns per extra call+accum-read).  DVE's dual-pass chunk
    costs ~4.9us vs ACT ~2.3us but starts later (waits for its bank
    pieces), so it gets 5 chunks to ACT's 13."""
    env = os.environ.get("SUPCON_DVE_CHUNKS")
    if env is not None:
        parts = env.split(";")
        return {t: {int(x) for x in parts[t].split(",") if x != ""}
                for t in range(NT)}
    b1, b2 = mk_b1 // W, mk_b2 // W
    d0 = {b1, b2}
    for cand in (7, 3, 0, 6, 1, 4):
        if len(d0) >= 3:
            break
        d0.add(cand)
    return {0: d0, 1: {b1, b2}}


def _emit_order(dvec):
    """Emission order of the 18 big chunks, chosen by a small pipeline
    model + hillclimb: 2 PSUM buffers rotate over allocations; a DVE
    chunk releases its buffer at pass-A end but pass A queues behind the
    previous pass B; piece arrivals follow the two-queue DMA plan."""
    arr_bb = 10.6
    arr_bk = [19.5, 16.9, 13.1, 20.1, 22.7, 13.7, 23.3, 16.3]

    def arrival(c):
        t, j = divmod(c, 9)
        return arr_bb if j == 0 else arr_bk[j - 1]

    def is_dve(c):
        t, j = divmod(c, 9)
        return j > 0 and (j - 1) in dvec[t]

    def sim(order, pe_t=1.0, act_t=2.3, dveA=2.4, dveB=2.5, start=7.0):
        pe = act = dve = start
        rel = [start, start]
        end = 0.0
        for i, c in enumerate(order):
            b = i % 2
            fd = max(pe, rel[b], arrival(c)) + pe_t
            pe = fd
            if is_dve(c):
                eA = max(dve, fd) + dveA
                dve = eA + dveB
                rel[b] = eA
                end = max(end, dve)
            else:
                e = max(act, fd) + act_t
                act = e
                rel[b] = e
                end = max(end, e)
        return end

    import random
    rng = random.Random(7)
    ids = list(range(18))
    best, bestm = None, None
    for restart in range(30):
        cur = rng.sample(ids, 18)
        curm = sim(cur)
        improved = True
        while improved:
            improved = False
            for a in range(18):
                for b in range(a + 1, 18):
                    o = cur[:]
                    o[a], o[b] = o[b], o[a]
                    m = sim(o)
                    if m < curm - 1e-9:
                        cur, curm = o, m
                        improved = True
        if bestm is None or curm < bestm:
            best, bestm = cur, curm
    return [divmod(c, 9) for c in best]


def _build(mk_b1, mk_b2):
    import ml_dtypes  # noqa: F401

    ops = _register_exp_ops()
    EXP_A = ops["EXP_POLY_A_ANT"]
    EXP_B = ops["EXP_SQ7_RED_ANT"]

    nc = bacc.Bacc()
    # anchT = [scaled anchors (APC) | unscaled anchors (APC) | g_c (C)]
    embT_d = nc.declare_dram_parameter("embT", [D, B], BF16, isOutput=False)
    anchT_d = nc.declare_dram_parameter("anchT", [D, 2 * APC + C], BF16, isOutput=False)
    bankT_d = nc.declare_dram_parameter("bankT", [D, M], BF16, isOutput=False)
    subs = _bank_subranges(mk_b1, mk_b2)
    NK = len(subs)
    dvec = _dve_chunks(mk_b1, mk_b2)
    # vecs: [invt | ninvt | invpc | coefv | cA | oneh | incl | kcol | eye]
    NV = NT * (5 + C + NK) + 1 + 128
    vecs_d = nc.declare_dram_parameter("vecs", [128, NV], F32, isOutput=False)
    oout_d = nc.declare_dram_parameter("oout", [128, 2 * NT], F32, isOutput=True)

    with tile.TileContext(nc) as tc:
        with (
            tc.tile_pool(name="big", bufs=1) as bigp,
            tc.tile_pool(name="sm", bufs=1) as smp,
            tc.tile_pool(name="ps", bufs=2, space="PSUM") as psp,
        ):
            anch_t = bigp.tile([D, 2 * APC + C], BF16, tag="anchT")
            vecs_t = smp.tile([128, NV], F32, tag="vecs")
            o = [0]

            def vslice(w):
                a = o[0]; o[0] += w
                return vecs_t[:, a:a + w]

            invt_t = vslice(NT)
            ninvt_t = vslice(NT)
            invpc_t = vslice(NT)
            coefv_t = vslice(NT)
            cA_t = vslice(NT)
            oneh_t = vslice(NT * C)
            incl_t = vslice(NT * NK)
            kcol_t = vslice(1)
            eye_t = vslice(128)

            emb_t = bigp.tile([D, B], BF16, tag="embT")
            bank_ts = [bigp.tile([D, W], BF16, tag=f"bank{j}", name=f"bank{j}")
                       for j in range(NBK)]
            H = B // 2
            # two HWDGE queues, pieces ordered by stream need: DVE-destined
            # pieces (2,5,7) first since the DVE chain is the long pole.
            # The scalar-engine issues happen while ACT waits for its first
            # data anyway (table load + emb transfer).
            def bkdma(eng, j):
                eng.dma_start(out=bank_ts[j][:], in_=bankT_d[:, j * W:(j + 1) * W])

            nc.sync.dma_start(out=vecs_t[:], in_=vecs_d[:])
            nc.scalar.dma_start(out=anch_t[:], in_=anchT_d[:])
            nc.sync.dma_start(out=emb_t[:, 0:H], in_=embT_d[:, 0:H])
            nc.scalar.dma_start(out=emb_t[:, H:B], in_=embT_d[:, H:B])
            for eng, j in [(nc.sync, 2), (nc.scalar, 5), (nc.sync, 7),
                           (nc.scalar, 1), (nc.sync, 0), (nc.scalar, 3),
                           (nc.sync, 4), (nc.scalar, 6)]:
                bkdma(eng, j)

            oout_t = smp.tile([128, 2 * NT], F32, tag="oout")
            scr_t = smp.tile([128, W], F32, tag="scrshared")
            wbuf_t = smp.tile([128, W], F32, tag="wbuf")
            dumb_t = smp.tile([128, W], F32, tag="dumb")
            sdiag = [smp.tile([128, 1], F32, tag=f"sdiag{t}", name=f"sdiag{t}") for t in range(NT)]
            selfe = [smp.tile([128, 1], F32, tag=f"selfe{t}", name=f"selfe{t}") for t in range(NT)]
            eyemul = smp.tile([128, 128], F32, tag="eyemul")
            warm = smp.tile([128, 1], F32, tag="warm")
            bbsum = [smp.tile([128, 1], F32, tag=f"bbsum{t}", name=f"bbsum{t}") for t in range(NT)]
            raw3 = [smp.tile([128, C], F32, tag=f"raw3{t}", name=f"raw3{t}") for t in range(NT)]
            esum = [smp.tile([128, NK], F32, tag=f"esum{t}", name=f"esum{t}") for t in range(NT)]

            # pull the Exp table load off the critical path
            nc.scalar.activation(warm[:], eye_t[:, 0:1], AF.Exp)

            def anch(t):
                return anch_t[:, t * 128:(t + 1) * 128]

            def anchu(t):
                return anch_t[:, APC + t * 128:APC + (t + 1) * 128]

            # ---- prelude: self blocks (scaled x unscaled) + positives ----
            pre_ps = psp.tile([128, W], F32, tag="chunk", name="pre_ps")
            for t in range(NT):
                nc.tensor.matmul(
                    pre_ps[:, t * 128:(t + 1) * 128], anch(t), anchu(t),
                    start=True, stop=True,
                )
            for t in range(NT):
                nc.tensor.matmul(
                    pre_ps[:, 256 + t * C:256 + (t + 1) * C], anch(t),
                    anch_t[:, 2 * APC:2 * APC + C], start=True, stop=True,
                )
            for t in range(NT):
                nc.vector.tensor_mul(eyemul[:], pre_ps[:, t * 128:(t + 1) * 128], eye_t[:])
                nc.vector.reduce_sum(sdiag[t][:], eyemul[:], axis=AX.X)
                nc.vector.tensor_copy(out=raw3[t][:], in_=pre_ps[:, 256 + t * C:256 + (t + 1) * C])
                nc.scalar.activation(
                    selfe[t][:], sdiag[t][:], AF.Exp,
                    bias=ninvt_t[:, t:t + 1], scale=ACT_SCALE,
                )

            by_chunk = {}
            for k, (s, e) in enumerate(subs):
                by_chunk.setdefault(s // W, []).append((s, e, k))

            scrNK = [smp.tile([128, NK], F32, tag=f"scrNK{t}", name=f"scrNK{t}") for t in range(NT)]
            scrC = [smp.tile([128, C], F32, tag=f"scrC{t}", name=f"scrC{t}") for t in range(NT)]

            def epi_early(t):
                """olin = coefv*invt*(1 - pos): prelude-only deps."""
                own_r = smp.tile([128, 1], F32, tag=f"ownr{t}", name=f"ownr{t}")
                pos = smp.tile([128, 1], F32, tag=f"pos{t}", name=f"pos{t}")
                w1 = smp.tile([128, 1], F32, tag=f"w1{t}", name=f"w1{t}")
                p1 = smp.tile([128, 1], F32, tag=f"p1{t}", name=f"p1{t}")
                nc.vector.tensor_mul(scrC[t][:], raw3[t][:], oneh_t[:, t * C:(t + 1) * C])
                nc.vector.reduce_sum(own_r[:], scrC[t][:], axis=AX.X)
                nc.vector.scalar_tensor_tensor(
                    out=pos[:], in0=own_r[:], scalar=sdiag[t][:], in1=invpc_t[:, t:t + 1],
                    op0=ALU.subtract, op1=ALU.mult,
                )
                nc.vector.scalar_tensor_tensor(
                    out=w1[:], in0=pos[:], scalar=-1.0, in1=invt_t[:, t:t + 1],
                    op0=ALU.mult, op1=ALU.mult,
                )
                nc.vector.scalar_tensor_tensor(
                    out=oout_t[:, NT + t:NT + t + 1], in0=w1[:], scalar=invt_t[:, t:t + 1],
                    in1=coefv_t[:, t:t + 1], op0=ALU.add, op1=ALU.mult,
                )
                return p1

            p1s = {}

            def epilogue(t):
                nc.vector.tensor_mul(scrNK[t][:], esum[t][:], incl_t[:, t * NK:(t + 1) * NK])
                nc.vector.reduce_sum(oout_t[:, t:t + 1], scrNK[t][:], axis=AX.X)
                nc.vector.tensor_add(oout_t[:, t:t + 1], oout_t[:, t:t + 1], p1s[t][:])

            def emit_bb(t):
                ps = psp.tile([128, W], F32, tag="chunk", name="bb_ps")
                for q in range(W // CH):
                    nc.tensor.matmul(
                        ps[:, q * CH:(q + 1) * CH], anch(t),
                        emb_t[:, q * CH:(q + 1) * CH],
                        start=True, stop=True,
                    )
                nc.scalar.activation(
                    scr_t[:], ps[:], AF.Exp,
                    bias=ninvt_t[:, t:t + 1], scale=ACT_SCALE,
                    accum_out=bbsum[t][:],
                )

            def emit_bank_act(t, j):
                ps = psp.tile([128, W], F32, tag="chunk", name="bk_ps")
                for q in range(W // CH):
                    nc.tensor.matmul(
                        ps[:, q * CH:(q + 1) * CH], anch(t),
                        bank_ts[j][:, q * CH:(q + 1) * CH],
                        start=True, stop=True,
                    )
                for (s, e, k) in by_chunk[j]:
                    a, b = s - j * W, e - j * W
                    nc.scalar.activation(
                        scr_t[:, a:b], ps[:, a:b], AF.Exp,
                        bias=ninvt_t[:, t:t + 1], scale=ACT_SCALE,
                        accum_out=esum[t][:, k:k + 1],
                    )

            dve_test = os.environ.get("SUPCON_DVE_TEST", "")

            def emit_bank_dve(t, j):
                ps = psp.tile([128, W], F32, tag="chunk", name="dk_ps")
                for q in range(W // CH):
                    nc.tensor.matmul(
                        ps[:, q * CH:(q + 1) * CH], anch(t),
                        bank_ts[j][:, q * CH:(q + 1) * CH],
                        start=True, stop=True,
                    )
                if dve_test == "stock":
                    # stock custom-dve op in place of pass A: wrong math,
                    # tests whether ANY custom-dve runs on this device
                    nc.vector.reciprocal_approx_fast(out=wbuf_t[:], in_=ps[:])
                else:
                    nc.vector._custom_dve(
                        EXP_A, out=wbuf_t[:], in0=ps[:],
                        s0=cA_t[:, t:t + 1], s1=EXP_BPK, imm2=EXP_GPK,
                    )
                for (s, e, k) in by_chunk[j]:
                    a, b = s - j * W, e - j * W
                    if dve_test in ("stock", "a"):
                        nc.vector.memset(esum[t][:, k:k + 1], 1.0)
                    else:
                        nc.vector._custom_dve(
                            EXP_B, out=dumb_t[:, a:b], in0=wbuf_t[:, a:b],
                            s0=0.0, accum_out=esum[t][:, k:k + 1],
                        )

            def emit_bank(t, j):
                if j in dvec[t]:
                    emit_bank_dve(t, j)
                else:
                    emit_bank_act(t, j)

            for t in range(NT):
                p1s[t] = epi_early(t)
            remaining = {t: NBK + 1 for t in range(NT)}
            for (t, j) in _emit_order(dvec):
                if j == 0:
                    emit_bb(t)
                    nc.vector.tensor_sub(p1s[t][:], bbsum[t][:], selfe[t][:])
                else:
                    emit_bank(t, j - 1)
                remaining[t] -= 1
                if remaining[t] == 0:
                    epilogue(t)

            nc.sync.dma_start(out=oout_d[:], in_=oout_t[:])

    nc.compile()
    return nc


def _per_core_cols(vec, core):
    """[B] host vector -> [128, NT] tile for one core (col t, partition p)."""
    sl = vec[core * APC:(core + 1) * APC]
    return np.ascontiguousarray(sl.reshape(NT, 128).T).astype(np.float32)


def kernel(embeddings, labels, bank_embs, bank_labels, class_temps):
    global LAST_EXEC_TIME_NS
    import ml_dtypes

    emb = np.asarray(embeddings, dtype=np.float32)
    bank = np.asarray(bank_embs, dtype=np.float32)
    lab = np.asarray(labels).astype(np.int64).ravel()
    blab = np.asarray(bank_labels).astype(np.int64).ravel()
    ct = np.asarray(class_temps, dtype=np.float32).ravel()

    bord = np.argsort(lab, kind="stable")
    slab = lab[bord]
    mord = np.argsort(blab, kind="stable")
    cnt = np.bincount(lab, minlength=C)
    mcnt = np.bincount(blab, minlength=C)
    mk_b1, mk_b2 = int(mcnt[0]), int(mcnt[0] + mcnt[1])

    embT = np.ascontiguousarray(emb[bord].T).astype(ml_dtypes.bfloat16)  # [D, B]
    bankT = np.ascontiguousarray(bank[mord].T).astype(ml_dtypes.bfloat16)  # [D, M]

    temps = ct[slab]
    inv_t = (1.0 / temps).astype(np.float32)
    pos_cnt = cnt[slab] - 1
    # positives matmul is scaled by inv_t/128 (pre-scaled anchors)
    invpc = (128.0 / KCBRT / inv_t / np.maximum(pos_cnt, 1)).astype(np.float32)
    validf = (pos_cnt > 0).astype(np.float32)
    coefv = (BASE_TEMP / temps).astype(np.float32) * validf
    oneh = np.eye(C, dtype=np.float32)[slab]      # [B, 3]
    n_valid = int((pos_cnt > 0).sum())

    nc = _build(mk_b1, mk_b2)

    subs = _bank_subranges(mk_b1, mk_b2)
    NK = len(subs)
    sub_cls = np.array([0 if s < mk_b1 else (1 if s < mk_b2 else 2) for s, _ in subs])
    incl_full = (sub_cls[None, :] != slab[:, None]).astype(np.float32)  # [B, NK]
    eye128 = np.eye(128, dtype=np.float32)

    # per-class embedding-sum vectors for the positives matmul (unscaled)
    gT = np.stack([emb[bord][slab == c].sum(axis=0) for c in range(C)], axis=1)
    gT = np.ascontiguousarray(gT).astype(ml_dtypes.bfloat16)

    # DVE pass A per-anchor shift: C0 = -inv_t/128 - u0
    cA = (KCBRT * (-inv_t / 128.0 - EXP_U0)).astype(np.float32)
    kcol = np.full((128, 1), EXP_K, dtype=np.float32)

    in_maps = []
    for core in range(NCORES):
        asl = slice(core * APC, (core + 1) * APC)
        oh = oneh[asl].reshape(NT, 128, C).transpose(1, 0, 2).reshape(128, NT * C)
        ic = incl_full[asl].reshape(NT, 128, NK).transpose(1, 0, 2).reshape(128, NT * NK)
        vecs = np.concatenate([
            _per_core_cols(inv_t, core),
            _per_core_cols(-inv_t, core),
            _per_core_cols(invpc, core),
            _per_core_cols(coefv, core),
            _per_core_cols(cA, core),
            oh.astype(np.float32),
            ic.astype(np.float32),
            kcol,
            eye128,
        ], axis=1)
        # scaled anchors: columns * inv_t_i/128 (scale BEFORE bf16 cast)
        anch_sc = (emb[bord][asl] * (inv_t[asl, None] / 128.0 * KCBRT)).T.astype(ml_dtypes.bfloat16)
        anch_un = embT[:, asl]
        in_maps.append({
            "embT": embT,
            "anchT": np.ascontiguousarray(
                np.concatenate([anch_sc, anch_un, gT], axis=1)),
            "bankT": bankT,
            "vecs": np.ascontiguousarray(vecs),
        })

    trace = os.environ.get("SUPCON_TRACE", "0") == "1"
    if trace:
        trace = _install_trace_shim()
    res = run_bass_kernel_spmd(nc, in_maps, core_ids=list(range(NCORES)), trace=trace)
    LAST_EXEC_TIME_NS = res.exec_time_ns

    # loss_i = coef_i * log(den_i) + lin_i; host finishes logs + masked mean
    loss_sum = np.float64(0.0)
    for core in range(NCORES):
        oo = np.asarray(res.results[core]["oout"], dtype=np.float64)    # [128, 2*NT]
        den, lin = oo[:, :NT], oo[:, NT:]
        cf = _per_core_cols(coefv, core).astype(np.float64)
        loss_sum += (cf * np.log(den) + lin).sum()
    return np.float32(loss_sum / max(n_valid, 1))


# revision 31
# speedup vs baseline: 1.0343x; 1.0343x over previous
"""ClassBalancedSupConLoss on 8 TRN2 NeuronCores (Bass/Tile), v2.

Math (reference semantics, reorganized for hardware):
  - All embeddings are unit-norm; fixed logsumexp shift m = 1:
        LSE_i = inv_t_i + log( sum_j exp(inv_t_i * (s_ij - 1)) )
    Self term excluded by subtracting exp of the bitwise-identical
    on-device s_ii product.  Batch and bank sorted by class on host so
    same-class columns are contiguous segments.
  - Anchors sharded 256/core across 8 cores; full embT/bankT replicas
    per core.  Device outputs per-anchor (den, lin); host does the
    2048 logs + masked mean.

v2 changes vs baseline (65.6us):
  - Input DMAs issued from sync+gpsimd queues only -- the scalar (ACT)
    engine previously spent ~9.5us issuing DMA descriptors before its
    first exp.
  - The exp stream is SPLIT between the ACT engine (hardware exp LUT,
    1 elem/lane/cyc @ 1.2GHz) and the Vector engine via two custom DVE
    ops: exp(z) ~= p3(z/128)^128 where p3 is a minimax cubic in
    factored form (pass A, 6 ALU slices) and pass B is 7 inline
    squarings with a free row-sum accumulator.  Max rel err 3.7e-4.
  - Anchor (stationary) operands are pre-scaled by inv_t/128 on host,
    so PSUM holds z/128 directly: ACT chunks use exp(128*x - inv_t)
    (free affine), DVE pass A needs only a per-partition shift.

SPMD: one program for all 8 cores; per-core data in the packed `vecs`
tile.
"""

import os
import numpy as np

import concourse.bass as bass  # noqa: F401
from concourse import bacc
import concourse.mybir as mybir
import concourse.tile as tile
from concourse.bass_utils import run_bass_kernel_spmd

B, D, M, C = 2048, 128, 16384, 3
NCORES = 8
APC = B // NCORES          # anchors per core = 256
NT = APC // 128            # anchor tiles per core = 2
CH = 512                   # matmul free chunk (one PSUM bank)
W = 2048                   # big PSUM chunk (4 banks) = one exp pass
NBK = M // W               # 8 bank pieces of [128, 2048]
BASE_TEMP = 0.07

F32 = mybir.dt.float32
BF16 = mybir.dt.bfloat16
AF = mybir.ActivationFunctionType
ALU = mybir.AluOpType
AX = mybir.AxisListType

LAST_EXEC_TIME_NS = None   # set by kernel() when SUPCON_TRACE=1

# ---- custom DVE exp: exp(z) = p3(v)^128, v = z/128 - u0 --------------------
# p3 fit of e^u on u in [-0.26, 0] (z in [-33, 0]; terms below e^-33 are
# ~1e-14 of the row sum).  Factored: p = (v*k) * ((v + bp)*v + gp),
# v = Src0 + C0 with C0 = -inv_t/128 - u0 per-anchor.  Pass B: w^128 via
# 7 squarings, row-sum accumulated in-instruction (no READ_ACCUMULATOR).
EXP_U0 = -1.7295465562795673
EXP_K = 0.146172629836262
EXP_BP = -1.791396476586659
EXP_GP = 4.062464246444453
# k absorbed into the variable: w = cbrt(k)*v, p = w*((w+BPk)*w+GPk);
# host scales anchors by inv_t/128*cbrt(k), ACT uses scale 128/cbrt(k)
KCBRT = float(EXP_K ** (1.0 / 3.0))
EXP_BPK = float(EXP_BP * KCBRT)
EXP_GPK = float(EXP_GP * KCBRT * KCBRT)
ACT_SCALE = float(128.0 / KCBRT)

_EXP_OPS = {}


def _register_exp_ops():
    """Define + register the two custom DVE ops with concourse's tables.

    dve_table_for_ops resolves op names through dve_ops.OPS /
    _SUB_OPCODE_FOR_NAME / CUSTOM_DVE_SPECS, all module-level registries;
    new ops just take the next free 5-bit opcode rows (18 in use < 32).
    """
    global _EXP_OPS
    if _EXP_OPS:
        return _EXP_OPS
    from operator import add as _add
    import concourse.dve_ops as dom
    from concourse.dve_spec import Spec, Src0, Src1, C0, C1, C2, lower, sq
    from concourse.dve_spec import _has_src1
    from concourse.dve_uop import DveOpSpec

    # pass A: q = v*((v+bp)*v+gp), v = Src0 + C0; the leading k is applied
    # in pass B's first slice (q*C1 before the squarings) -- keeps pass A
    # at three constant slots with no Src1 stream.
    tmode = os.environ.get("SUPCON_DVE_SPEC", "fan3")
    if tmode == "fan2":
        va = Src0 + C0
        vb = Src0 + C0
        body_a = ((va + C1) * va + C2) * vb
    else:
        v = Src0 + C0
        body_a = v * ((v + C1) * v + C2)

    def ref_a(in0, in1, s0, s1, imm2):
        vv = in0.astype(np.float32) + np.asarray(s0, np.float32)
        return (vv * ((vv + s1) * vv + imm2)).astype(np.float32)

    body_b = sq(sq(sq(sq(sq(sq(sq(Src0)))))))

    def ref_b(in0, in1, s0, s1, imm2):
        b = in0.astype(np.float32)
        for _ in range(7):
            b = (b * b).astype(np.float32)
        acc = np.asarray(s0, np.float32).reshape(-1, 1) + b.reshape(
            b.shape[0], -1).sum(axis=-1, keepdims=True)
        return b, acc.astype(np.float32)

    spec_a = Spec(body=body_a, reference=ref_a)
    spec_b = Spec(body=body_b, accum=_add, accum_init=C0, reference=ref_b)

    # The DVE NX firmware dispatch table only knows the stock opcode rows,
    # so new rows would hang the engine.  Instead REPLACE the table
    # programs of two stock ops this kernel never calls -- the per-NEFF
    # uop table (qDveTable) is regenerated from dve_ops.OPS at compile
    # time, so the hijacked rows carry the exp programs.
    hijack = {"EXP_POLY_A_ANT": "CODY_WAITE_CASCADE",
              "EXP_SQ7_RED_ANT": "ADD_RANGE_WRAP"}
    for myname, spec in [("EXP_POLY_A_ANT", spec_a), ("EXP_SQ7_RED_ANT", spec_b)]:
        name = hijack[myname]
        idx = next(i for i, o in enumerate(dom.OPS) if o.name == name)
        if dom.OPS[idx].spec is spec:
            _EXP_OPS[myname] = dom.OPS[idx]
            continue
        row = dom._SUB_OPCODE_FOR_NAME[name]
        shas = {}
        for ver in ("v3", "v4"):
            try:
                r = DveOpSpec(name=name, opcode=row, uops=lower(spec, ver=ver),
                              rd1_en=_has_src1(spec))
                shas[ver] = r.sha(ver)
            except Exception:
                pass
        op = dom.DveOp(name, spec, subdim=False, uops_sha=shas)
        dom.OPS[idx] = op
        dom.CUSTOM_DVE_SPECS[name] = spec
        _EXP_OPS[myname] = op
    return _EXP_OPS


def _install_trace_shim():
    """Register the NTFF profile hook that this image's antenv lacks."""
    import sys
    import types
    import ctypes
    import contextlib

    try:
        from antenv.axon_hooks import get_axon_ntff_profile_hook  # noqa: F401
        return True
    except ImportError:
        pass

    so_path = "/opt/axon/libaxon_pjrt.so"
    if not os.path.exists(so_path):
        return False
    lib = ctypes.CDLL(so_path)
    if not hasattr(lib, "axon_start_nrt_profile"):
        return False
    lib.axon_start_nrt_profile.argtypes = [
        ctypes.POINTER(ctypes.c_int64),
        ctypes.c_size_t,
    ]
    lib.axon_start_nrt_profile.restype = ctypes.c_int64
    lib.axon_stop_nrt_profile.argtypes = [ctypes.c_char_p]
    lib.axon_stop_nrt_profile.restype = ctypes.c_int64

    @contextlib.contextmanager
    def _hook(output_dir, device_ids):
        import jax

        jax.devices()
        if device_ids:
            ids = (ctypes.c_int64 * len(device_ids))(*device_ids)
            rc = lib.axon_start_nrt_profile(ids, len(device_ids))
        else:
            rc = lib.axon_start_nrt_profile(None, 0)
        if rc != 0:
            raise RuntimeError(f"axon_start_nrt_profile rc={rc}")
        try:
            yield
        finally:
            n = lib.axon_stop_nrt_profile(str(output_dir).encode())
            print(f"profile: {n} file(s) written to {output_dir}", file=sys.stderr)

    _state = {"hook": _hook}
    mod = types.ModuleType("antenv.axon_hooks")
    mod.get_axon_ntff_profile_hook = lambda: _state["hook"]
    mod.set_axon_ntff_profile_hook = lambda h: _state.update(hook=h)
    sys.modules["antenv.axon_hooks"] = mod
    import antenv

    antenv.axon_hooks = mod

    import concourse.bass_utils as bu

    bu.upload_artifacts = lambda tmpdir: tmpdir
    return True


def _bank_subranges(mk_b1, mk_b2):
    """Split [0, M) at big-chunk multiples AND class boundaries."""
    cuts = sorted({c * W for c in range(NBK + 1)} | {mk_b1, mk_b2})
    subs = [(cuts[i], cuts[i + 1]) for i in range(len(cuts) - 1)]
    return subs


def _dve_chunks(mk_b1, mk_b2):
    """Per-tile bank chunk sets handled by the Vector engine (rest ACT).

    Boundary-containing chunks go to DVE (its per-call overhead is tiny,
    ACT pays ~600ns per extra call+accum-read).  DVE's dual-pass chunk
    costs ~4.9us vs ACT ~2.3us but starts later (waits for its bank
    pieces), so it gets 5 chunks to ACT's 13."""
    env = os.environ.get("SUPCON_DVE_CHUNKS")
    if env is not None:
        parts = env.split(";")
        return {t: {int(x) for x in parts[t].split(",") if x != ""}
                for t in range(NT)}
    b1, b2 = mk_b1 // W, mk_b2 // W
    d0 = {b1, b2}
    for cand in (7, 3, 0, 6, 1, 4):
        if len(d0) >= 3:
            break
        d0.add(cand)
    return {0: d0, 1: {b1, b2}}


def _emit_order(dvec):
    """Emission order of the 18 big chunks, chosen by a small pipeline
    model + hillclimb: 2 PSUM buffers rotate over allocations; a DVE
    chunk releases its buffer at pass-A end but pass A queues behind the
    previous pass B; piece arrivals follow the two-queue DMA plan."""
    arr_bb = 11.0
    arr_bk = [21.0, 17.4, 13.9, 21.7, 24.6, 14.6, 25.3, 18.1]

    def arrival(c):
        t, j = divmod(c, 9)
        return arr_bb if j == 0 else arr_bk[j - 1]

    def is_dve(c):
        t, j = divmod(c, 9)
        return j > 0 and (j - 1) in dvec[t]

    def sim(order, pe_t=1.0, act_t=2.3, dveA=2.4, dveB=2.5, start=7.0):
        pe = act = dve = start
        rel = [start, start]
        end = 0.0
        for i, c in enumerate(order):
            b = i % 2
            fd = max(pe, rel[b], arrival(c)) + pe_t
            pe = fd
            if is_dve(c):
                eA = max(dve, fd) + dveA
                dve = eA + dveB
                rel[b] = eA
                end = max(end, dve)
            else:
                e = max(act, fd) + act_t
                act = e
                rel[b] = e
                end = max(end, e)
        return end

    import random
    rng = random.Random(7)
    ids = list(range(18))
    best, bestm = None, None
    for restart in range(30):
        cur = rng.sample(ids, 18)
        curm = sim(cur)
        improved = True
        while improved:
            improved = False
            for a in range(18):
                for b in range(a + 1, 18):
                    o = cur[:]
                    o[a], o[b] = o[b], o[a]
                    m = sim(o)
                    if m < curm - 1e-9:
                        cur, curm = o, m
                        improved = True
        if bestm is None or curm < bestm:
            best, bestm = cur, curm
    return [divmod(c, 9) for c in best]


def _build(mk_b1, mk_b2):
    import ml_dtypes  # noqa: F401

    ops = _register_exp_ops()
    EXP_A = ops["EXP_POLY_A_ANT"]
    EXP_B = ops["EXP_SQ7_RED_ANT"]

    nc = bacc.Bacc()
    # anchT = [scaled anchors (APC) | unscaled anchors (APC) | g_c (C)]
    embT_d = nc.declare_dram_parameter("embT", [D, B], BF16, isOutput=False)
    anchT_d = nc.declare_dram_parameter("anchT", [D, 2 * APC + C], BF16, isOutput=False)
    bankT_d = nc.declare_dram_parameter("bankT", [D, M], BF16, isOutput=False)
    subs = _bank_subranges(mk_b1, mk_b2)
    NK = len(subs)
    dvec = _dve_chunks(mk_b1, mk_b2)
    # vecs: [invt | ninvt | invpc | coefv | cA | oneh | incl | kcol | eye]
    NV = NT * (5 + C + NK) + 1 + 128
    vecs_d = nc.declare_dram_parameter("vecs", [128, NV], F32, isOutput=False)
    oout_d = nc.declare_dram_parameter("oout", [128, 2 * NT], F32, isOutput=True)

    with tile.TileContext(nc) as tc:
        with (
            tc.tile_pool(name="big", bufs=1) as bigp,
            tc.tile_pool(name="sm", bufs=1) as smp,
            tc.tile_pool(name="ps", bufs=2, space="PSUM") as psp,
        ):
            anch_t = bigp.tile([D, 2 * APC + C], BF16, tag="anchT")
            vecs_t = smp.tile([128, NV], F32, tag="vecs")
            junkw_t = bigp.tile([128, 128], BF16, tag="junkw")
            junkx_t = bigp.tile([128, CH], BF16, tag="junkx")
            o = [0]

            def vslice(w):
                a = o[0]; o[0] += w
                return vecs_t[:, a:a + w]

            invt_t = vslice(NT)
            ninvt_t = vslice(NT)
            invpc_t = vslice(NT)
            coefv_t = vslice(NT)
            cA_t = vslice(NT)
            oneh_t = vslice(NT * C)
            incl_t = vslice(NT * NK)
            kcol_t = vslice(1)
            eye_t = vslice(128)

            emb_t = bigp.tile([D, B], BF16, tag="embT")
            bank_ts = [bigp.tile([D, W], BF16, tag=f"bank{j}", name=f"bank{j}")
                       for j in range(NBK)]
            H = B // 2
            # two HWDGE queues, pieces ordered by stream need: DVE-destined
            # pieces (2,5,7) first since the DVE chain is the long pole.
            # The scalar-engine issues happen while ACT waits for its first
            # data anyway (table load + emb transfer).
            def bkdma(eng, j):
                eng.dma_start(out=bank_ts[j][:], in_=bankT_d[:, j * W:(j + 1) * W])

            nc.sync.dma_start(out=vecs_t[:], in_=vecs_d[:])
            nc.scalar.dma_start(out=anch_t[:], in_=anchT_d[:])
            nc.sync.dma_start(out=emb_t[:, 0:H], in_=embT_d[:, 0:H])
            nc.scalar.dma_start(out=emb_t[:, H:B], in_=embT_d[:, H:B])
            for eng, j in [(nc.sync, 2), (nc.scalar, 5), (nc.sync, 1),
                           (nc.scalar, 7), (nc.sync, 0), (nc.scalar, 3),
                           (nc.sync, 4), (nc.scalar, 6)]:
                bkdma(eng, j)

            oout_t = smp.tile([128, 2 * NT], F32, tag="oout")
            scr_t = smp.tile([128, W], F32, tag="scrshared")
            wbuf_t = smp.tile([128, W], F32, tag="wbuf")
            dumb_t = smp.tile([128, W], F32, tag="dumb")
            sdiag = [smp.tile([128, 1], F32, tag=f"sdiag{t}", name=f"sdiag{t}") for t in range(NT)]
            selfe = [smp.tile([128, 1], F32, tag=f"selfe{t}", name=f"selfe{t}") for t in range(NT)]
            eyemul = smp.tile([128, 128], F32, tag="eyemul")
            warm = smp.tile([128, 1], F32, tag="warm")
            bbsum = [smp.tile([128, 1], F32, tag=f"bbsum{t}", name=f"bbsum{t}") for t in range(NT)]
            raw3 = [smp.tile([128, C], F32, tag=f"raw3{t}", name=f"raw3{t}") for t in range(NT)]
            esum = [smp.tile([128, NK], F32, tag=f"esum{t}", name=f"esum{t}") for t in range(NT)]

            # pull the Exp table load off the critical path
            nc.scalar.activation(warm[:], eye_t[:, 0:1], AF.Exp)

            def anch(t):
                return anch_t[:, t * 128:(t + 1) * 128]

            def anchu(t):
                return anch_t[:, APC + t * 128:APC + (t + 1) * 128]

            # HAM warmup: PE activity from program start so the clock is at
            # 2.4GHz when the stream begins (junk operands, results unread;
            # memsets on gpsimd so the vector engine isn't delayed)
            nc.gpsimd.memset(junkw_t[:], 0.0)
            nc.gpsimd.memset(junkx_t[:], 0.0)
            warm_ps = psp.tile([128, W], F32, tag="chunk", name="warm_ps")
            for w in range(5):
                nc.tensor.matmul(
                    warm_ps[:, (w % 4) * CH:((w % 4) + 1) * CH],
                    junkw_t[:], junkx_t[:], start=True, stop=True,
                )

            # ---- prelude: self blocks (scaled x unscaled) + positives ----
            pre_ps = psp.tile([128, W], F32, tag="chunk", name="pre_ps")
            for t in range(NT):
                nc.tensor.matmul(
                    pre_ps[:, t * 128:(t + 1) * 128], anch(t), anchu(t),
                    start=True, stop=True,
                )
            for t in range(NT):
                nc.tensor.matmul(
                    pre_ps[:, 256 + t * C:256 + (t + 1) * C], anch(t),
                    anch_t[:, 2 * APC:2 * APC + C], start=True, stop=True,
                )
            for t in range(NT):
                nc.vector.tensor_mul(eyemul[:], pre_ps[:, t * 128:(t + 1) * 128], eye_t[:])
                nc.vector.reduce_sum(sdiag[t][:], eyemul[:], axis=AX.X)
                nc.vector.tensor_copy(out=raw3[t][:], in_=pre_ps[:, 256 + t * C:256 + (t + 1) * C])
                nc.scalar.activation(
                    selfe[t][:], sdiag[t][:], AF.Exp,
                    bias=ninvt_t[:, t:t + 1], scale=ACT_SCALE,
                )

            by_chunk = {}
            for k, (s, e) in enumerate(subs):
                by_chunk.setdefault(s // W, []).append((s, e, k))

            scrNK = [smp.tile([128, NK], F32, tag=f"scrNK{t}", name=f"scrNK{t}") for t in range(NT)]
            scrC = [smp.tile([128, C], F32, tag=f"scrC{t}", name=f"scrC{t}") for t in range(NT)]

            def epi_early(t):
                """olin = coefv*invt*(1 - pos): prelude-only deps."""
                own_r = smp.tile([128, 1], F32, tag=f"ownr{t}", name=f"ownr{t}")
                pos = smp.tile([128, 1], F32, tag=f"pos{t}", name=f"pos{t}")
                w1 = smp.tile([128, 1], F32, tag=f"w1{t}", name=f"w1{t}")
                p1 = smp.tile([128, 1], F32, tag=f"p1{t}", name=f"p1{t}")
                nc.vector.tensor_mul(scrC[t][:], raw3[t][:], oneh_t[:, t * C:(t + 1) * C])
                nc.vector.reduce_sum(own_r[:], scrC[t][:], axis=AX.X)
                nc.vector.scalar_tensor_tensor(
                    out=pos[:], in0=own_r[:], scalar=sdiag[t][:], in1=invpc_t[:, t:t + 1],
                    op0=ALU.subtract, op1=ALU.mult,
                )
                nc.vector.scalar_tensor_tensor(
                    out=w1[:], in0=pos[:], scalar=-1.0, in1=invt_t[:, t:t + 1],
                    op0=ALU.mult, op1=ALU.mult,
                )
                nc.vector.scalar_tensor_tensor(
                    out=oout_t[:, NT + t:NT + t + 1], in0=w1[:], scalar=invt_t[:, t:t + 1],
                    in1=coefv_t[:, t:t + 1], op0=ALU.add, op1=ALU.mult,
                )
                return p1

            p1s = {}

            def epilogue(t):
                nc.vector.tensor_mul(scrNK[t][:], esum[t][:], incl_t[:, t * NK:(t + 1) * NK])
                nc.vector.reduce_sum(oout_t[:, t:t + 1], scrNK[t][:], axis=AX.X)
                nc.vector.tensor_add(oout_t[:, t:t + 1], oout_t[:, t:t + 1], p1s[t][:])

            def emit_bb(t):
                ps = psp.tile([128, W], F32, tag="chunk", name="bb_ps")
                for q in range(W // CH):
                    nc.tensor.matmul(
                        ps[:, q * CH:(q + 1) * CH], anch(t),
                        emb_t[:, q * CH:(q + 1) * CH],
                        start=True, stop=True,
                    )
                nc.scalar.activation(
                    scr_t[:], ps[:], AF.Exp,
                    bias=ninvt_t[:, t:t + 1], scale=ACT_SCALE,
                    accum_out=bbsum[t][:],
                )

            def emit_bank_act(t, j):
                ps = psp.tile([128, W], F32, tag="chunk", name="bk_ps")
                for q in range(W // CH):
                    nc.tensor.matmul(
                        ps[:, q * CH:(q + 1) * CH], anch(t),
                        bank_ts[j][:, q * CH:(q + 1) * CH],
                        start=True, stop=True,
                    )
                for (s, e, k) in by_chunk[j]:
                    a, b = s - j * W, e - j * W
                    nc.scalar.activation(
                        scr_t[:, a:b], ps[:, a:b], AF.Exp,
                        bias=ninvt_t[:, t:t + 1], scale=ACT_SCALE,
                        accum_out=esum[t][:, k:k + 1],
                    )

            dve_test = os.environ.get("SUPCON_DVE_TEST", "")

            def emit_bank_dve(t, j):
                ps = psp.tile([128, W], F32, tag="chunk", name="dk_ps")
                for q in range(W // CH):
                    nc.tensor.matmul(
                        ps[:, q * CH:(q + 1) * CH], anch(t),
                        bank_ts[j][:, q * CH:(q + 1) * CH],
                        start=True, stop=True,
                    )
                if dve_test == "stock":
                    # stock custom-dve op in place of pass A: wrong math,
                    # tests whether ANY custom-dve runs on this device
                    nc.vector.reciprocal_approx_fast(out=wbuf_t[:], in_=ps[:])
                else:
                    nc.vector._custom_dve(
                        EXP_A, out=wbuf_t[:], in0=ps[:],
                        s0=cA_t[:, t:t + 1], s1=EXP_BPK, imm2=EXP_GPK,
                    )
                for (s, e, k) in by_chunk[j]:
                    a, b = s - j * W, e - j * W
                    if dve_test in ("stock", "a"):
                        nc.vector.memset(esum[t][:, k:k + 1], 1.0)
                    else:
                        nc.vector._custom_dve(
                            EXP_B, out=dumb_t[:, a:b], in0=wbuf_t[:, a:b],
                            s0=0.0, accum_out=esum[t][:, k:k + 1],
                        )

            def emit_bank(t, j):
                if j in dvec[t]:
                    emit_bank_dve(t, j)
                else:
                    emit_bank_act(t, j)

            for t in range(NT):
                p1s[t] = epi_early(t)
            remaining = {t: NBK + 1 for t in range(NT)}
            for (t, j) in _emit_order(dvec):
                if j == 0:
                    emit_bb(t)
                    nc.vector.tensor_sub(p1s[t][:], bbsum[t][:], selfe[t][:])
                else:
                    emit_bank(t, j - 1)
                remaining[t] -= 1
                if remaining[t] == 0:
                    epilogue(t)

            nc.sync.dma_start(out=oout_d[:], in_=oout_t[:])

    nc.compile()
    return nc


def _per_core_cols(vec, core):
    """[B] host vector -> [128, NT] tile for one core (col t, partition p)."""
    sl = vec[core * APC:(core + 1) * APC]
    return np.ascontiguousarray(sl.reshape(NT, 128).T).astype(np.float32)


def kernel(embeddings, labels, bank_embs, bank_labels, class_temps):
    global LAST_EXEC_TIME_NS
    import ml_dtypes

    emb = np.asarray(embeddings, dtype=np.float32)
    bank = np.asarray(bank_embs, dtype=np.float32)
    lab = np.asarray(labels).astype(np.int64).ravel()
    blab = np.asarray(bank_labels).astype(np.int64).ravel()
    ct = np.asarray(class_temps, dtype=np.float32).ravel()

    bord = np.argsort(lab, kind="stable")
    slab = lab[bord]
    mord = np.argsort(blab, kind="stable")
    cnt = np.bincount(lab, minlength=C)
    mcnt = np.bincount(blab, minlength=C)
    mk_b1, mk_b2 = int(mcnt[0]), int(mcnt[0] + mcnt[1])

    embT = np.ascontiguousarray(emb[bord].T).astype(ml_dtypes.bfloat16)  # [D, B]
    bankT = np.ascontiguousarray(bank[mord].T).astype(ml_dtypes.bfloat16)  # [D, M]

    temps = ct[slab]
    inv_t = (1.0 / temps).astype(np.float32)
    pos_cnt = cnt[slab] - 1
    # positives matmul is scaled by inv_t/128 (pre-scaled anchors)
    invpc = (128.0 / KCBRT / inv_t / np.maximum(pos_cnt, 1)).astype(np.float32)
    validf = (pos_cnt > 0).astype(np.float32)
    coefv = (BASE_TEMP / temps).astype(np.float32) * validf
    oneh = np.eye(C, dtype=np.float32)[slab]      # [B, 3]
    n_valid = int((pos_cnt > 0).sum())

    nc = _build(mk_b1, mk_b2)

    subs = _bank_subranges(mk_b1, mk_b2)
    NK = len(subs)
    sub_cls = np.array([0 if s < mk_b1 else (1 if s < mk_b2 else 2) for s, _ in subs])
    incl_full = (sub_cls[None, :] != slab[:, None]).astype(np.float32)  # [B, NK]
    eye128 = np.eye(128, dtype=np.float32)

    # per-class embedding-sum vectors for the positives matmul (unscaled)
    gT = np.stack([emb[bord][slab == c].sum(axis=0) for c in range(C)], axis=1)
    gT = np.ascontiguousarray(gT).astype(ml_dtypes.bfloat16)

    # DVE pass A per-anchor shift: C0 = -inv_t/128 - u0
    cA = (KCBRT * (-inv_t / 128.0 - EXP_U0)).astype(np.float32)
    kcol = np.full((128, 1), EXP_K, dtype=np.float32)

    in_maps = []
    for core in range(NCORES):
        asl = slice(core * APC, (core + 1) * APC)
        oh = oneh[asl].reshape(NT, 128, C).transpose(1, 0, 2).reshape(128, NT * C)
        ic = incl_full[asl].reshape(NT, 128, NK).transpose(1, 0, 2).reshape(128, NT * NK)
        vecs = np.concatenate([
            _per_core_cols(inv_t, core),
            _per_core_cols(-inv_t, core),
            _per_core_cols(invpc, core),
            _per_core_cols(coefv, core),
            _per_core_cols(cA, core),
            oh.astype(np.float32),
            ic.astype(np.float32),
            kcol,
            eye128,
        ], axis=1)
        # scaled anchors: columns * inv_t_i/128 (scale BEFORE bf16 cast)
        anch_sc = (emb[bord][asl] * (inv_t[asl, None] / 128.0 * KCBRT)).T.astype(ml_dtypes.bfloat16)
        anch_un = embT[:, asl]
        in_maps.append({
            "embT": embT,
            "anchT": np.ascontiguousarray(
                np.concatenate([anch_sc, anch_un, gT], axis=1)),
            "bankT": bankT,
            "vecs": np.ascontiguousarray(vecs),
        })

    trace = os.environ.get("SUPCON_TRACE", "0") == "1"
    if trace:
        trace = _install_trace_shim()
    res = run_bass_kernel_spmd(nc, in_maps, core_ids=list(range(NCORES)), trace=trace)
    LAST_EXEC_TIME_NS = res.exec_time_ns

    # loss_i = coef_i * log(den_i) + lin_i; host finishes logs + masked mean
    loss_sum = np.float64(0.0)
    for core in range(NCORES):
        oo = np.asarray(res.results[core]["oout"], dtype=np.float64)    # [128, 2*NT]
        den, lin = oo[:, :NT], oo[:, NT:]
        cf = _per_core_cols(coefv, core).astype(np.float64)
        loss_sum += (cf * np.log(den) + lin).sum()
    return np.float32(loss_sum / max(n_valid, 1))


# revision 36
# speedup vs baseline: 1.0526x; 1.0177x over previous
"""ClassBalancedSupConLoss on 8 TRN2 NeuronCores (Bass/Tile), v2.

Math (reference semantics, reorganized for hardware):
  - All embeddings are unit-norm; fixed logsumexp shift m = 1:
        LSE_i = inv_t_i + log( sum_j exp(inv_t_i * (s_ij - 1)) )
    Self term excluded by subtracting exp of the bitwise-identical
    on-device s_ii product.  Batch and bank sorted by class on host so
    same-class columns are contiguous segments.
  - Anchors sharded 256/core across 8 cores; full embT/bankT replicas
    per core.  Device outputs per-anchor (den, lin); host does the
    2048 logs + masked mean.

v2 changes vs baseline (65.6us):
  - Input DMAs issued from sync+gpsimd queues only -- the scalar (ACT)
    engine previously spent ~9.5us issuing DMA descriptors before its
    first exp.
  - The exp stream is SPLIT between the ACT engine (hardware exp LUT,
    1 elem/lane/cyc @ 1.2GHz) and the Vector engine via two custom DVE
    ops: exp(z) ~= p3(z/128)^128 where p3 is a minimax cubic in
    factored form (pass A, 6 ALU slices) and pass B is 7 inline
    squarings with a free row-sum accumulator.  Max rel err 3.7e-4.
  - Anchor (stationary) operands are pre-scaled by inv_t/128 on host,
    so PSUM holds z/128 directly: ACT chunks use exp(128*x - inv_t)
    (free affine), DVE pass A needs only a per-partition shift.

SPMD: one program for all 8 cores; per-core data in the packed `vecs`
tile.
"""

import os
import numpy as np

import concourse.bass as bass  # noqa: F401
from concourse import bacc
import concourse.mybir as mybir
import concourse.tile as tile
from concourse.bass_utils import run_bass_kernel_spmd

B, D, M, C = 2048, 128, 16384, 3
# bank piece storage order in DRAM: slots 0-3 single sync DMAs, slots
# 4-5 and 6-7 merged scalar DMAs
BPERM = [2, 1, 0, 4, 5, 7, 3, 6]
NCORES = 8
APC = B // NCORES          # anchors per core = 256
NT = APC // 128            # anchor tiles per core = 2
CH = 512                   # matmul free chunk (one PSUM bank)
W = 2048                   # big PSUM chunk (4 banks) = one exp pass
NBK = M // W               # 8 bank pieces of [128, 2048]
BASE_TEMP = 0.07

F32 = mybir.dt.float32
BF16 = mybir.dt.bfloat16
AF = mybir.ActivationFunctionType
ALU = mybir.AluOpType
AX = mybir.AxisListType

LAST_EXEC_TIME_NS = None   # set by kernel() when SUPCON_TRACE=1

# ---- custom DVE exp: exp(z) = p3(v)^128, v = z/128 - u0 --------------------
# p3 fit of e^u on u in [-0.26, 0] (z in [-33, 0]; terms below e^-33 are
# ~1e-14 of the row sum).  Factored: p = (v*k) * ((v + bp)*v + gp),
# v = Src0 + C0 with C0 = -inv_t/128 - u0 per-anchor.  Pass B: w^128 via
# 7 squarings, row-sum accumulated in-instruction (no READ_ACCUMULATOR).
EXP_U0 = -1.7295465562795673
EXP_K = 0.146172629836262
EXP_BP = -1.791396476586659
EXP_GP = 4.062464246444453
# k absorbed into the variable: w = cbrt(k)*v, p = w*((w+BPk)*w+GPk);
# host scales anchors by inv_t/128*cbrt(k), ACT uses scale 128/cbrt(k)
KCBRT = float(EXP_K ** (1.0 / 3.0))
EXP_BPK = float(EXP_BP * KCBRT)
EXP_GPK = float(EXP_GP * KCBRT * KCBRT)
ACT_SCALE = float(128.0 / KCBRT)

_EXP_OPS = {}


def _register_exp_ops():
    """Define + register the two custom DVE ops with concourse's tables.

    dve_table_for_ops resolves op names through dve_ops.OPS /
    _SUB_OPCODE_FOR_NAME / CUSTOM_DVE_SPECS, all module-level registries;
    new ops just take the next free 5-bit opcode rows (18 in use < 32).
    """
    global _EXP_OPS
    if _EXP_OPS:
        return _EXP_OPS
    from operator import add as _add
    import concourse.dve_ops as dom
    from concourse.dve_spec import Spec, Src0, Src1, C0, C1, C2, lower, sq
    from concourse.dve_spec import _has_src1
    from concourse.dve_uop import DveOpSpec

    # pass A: q = v*((v+bp)*v+gp), v = Src0 + C0; the leading k is applied
    # in pass B's first slice (q*C1 before the squarings) -- keeps pass A
    # at three constant slots with no Src1 stream.
    tmode = os.environ.get("SUPCON_DVE_SPEC", "fan3")
    if tmode == "fan2":
        va = Src0 + C0
        vb = Src0 + C0
        body_a = ((va + C1) * va + C2) * vb
    else:
        v = Src0 + C0
        body_a = v * ((v + C1) * v + C2)

    def ref_a(in0, in1, s0, s1, imm2):
        vv = in0.astype(np.float32) + np.asarray(s0, np.float32)
        return (vv * ((vv + s1) * vv + imm2)).astype(np.float32)

    body_b = sq(sq(sq(sq(sq(sq(sq(Src0)))))))

    def ref_b(in0, in1, s0, s1, imm2):
        b = in0.astype(np.float32)
        for _ in range(7):
            b = (b * b).astype(np.float32)
        acc = np.asarray(s0, np.float32).reshape(-1, 1) + b.reshape(
            b.shape[0], -1).sum(axis=-1, keepdims=True)
        return b, acc.astype(np.float32)

    spec_a = Spec(body=body_a, reference=ref_a)
    spec_b = Spec(body=body_b, accum=_add, accum_init=C0, reference=ref_b)

    # The DVE NX firmware dispatch table only knows the stock opcode rows,
    # so new rows would hang the engine.  Instead REPLACE the table
    # programs of two stock ops this kernel never calls -- the per-NEFF
    # uop table (qDveTable) is regenerated from dve_ops.OPS at compile
    # time, so the hijacked rows carry the exp programs.
    hijack = {"EXP_POLY_A_ANT": "CODY_WAITE_CASCADE",
              "EXP_SQ7_RED_ANT": "ADD_RANGE_WRAP"}
    for myname, spec in [("EXP_POLY_A_ANT", spec_a), ("EXP_SQ7_RED_ANT", spec_b)]:
        name = hijack[myname]
        idx = next(i for i, o in enumerate(dom.OPS) if o.name == name)
        if dom.OPS[idx].spec is spec:
            _EXP_OPS[myname] = dom.OPS[idx]
            continue
        row = dom._SUB_OPCODE_FOR_NAME[name]
        shas = {}
        for ver in ("v3", "v4"):
            try:
                r = DveOpSpec(name=name, opcode=row, uops=lower(spec, ver=ver),
                              rd1_en=_has_src1(spec))
                shas[ver] = r.sha(ver)
            except Exception:
                pass
        op = dom.DveOp(name, spec, subdim=False, uops_sha=shas)
        dom.OPS[idx] = op
        dom.CUSTOM_DVE_SPECS[name] = spec
        _EXP_OPS[myname] = op
    return _EXP_OPS


def _install_trace_shim():
    """Register the NTFF profile hook that this image's antenv lacks."""
    import sys
    import types
    import ctypes
    import contextlib

    try:
        from antenv.axon_hooks import get_axon_ntff_profile_hook  # noqa: F401
        return True
    except ImportError:
        pass

    so_path = "/opt/axon/libaxon_pjrt.so"
    if not os.path.exists(so_path):
        return False
    lib = ctypes.CDLL(so_path)
    if not hasattr(lib, "axon_start_nrt_profile"):
        return False
    lib.axon_start_nrt_profile.argtypes = [
        ctypes.POINTER(ctypes.c_int64),
        ctypes.c_size_t,
    ]
    lib.axon_start_nrt_profile.restype = ctypes.c_int64
    lib.axon_stop_nrt_profile.argtypes = [ctypes.c_char_p]
    lib.axon_stop_nrt_profile.restype = ctypes.c_int64

    @contextlib.contextmanager
    def _hook(output_dir, device_ids):
        import jax

        jax.devices()
        if device_ids:
            ids = (ctypes.c_int64 * len(device_ids))(*device_ids)
            rc = lib.axon_start_nrt_profile(ids, len(device_ids))
        else:
            rc = lib.axon_start_nrt_profile(None, 0)
        if rc != 0:
            raise RuntimeError(f"axon_start_nrt_profile rc={rc}")
        try:
            yield
        finally:
            n = lib.axon_stop_nrt_profile(str(output_dir).encode())
            print(f"profile: {n} file(s) written to {output_dir}", file=sys.stderr)

    _state = {"hook": _hook}
    mod = types.ModuleType("antenv.axon_hooks")
    mod.get_axon_ntff_profile_hook = lambda: _state["hook"]
    mod.set_axon_ntff_profile_hook = lambda h: _state.update(hook=h)
    sys.modules["antenv.axon_hooks"] = mod
    import antenv

    antenv.axon_hooks = mod

    import concourse.bass_utils as bu

    bu.upload_artifacts = lambda tmpdir: tmpdir
    return True


def _bank_subranges(mk_b1, mk_b2):
    """Split [0, M) at big-chunk multiples AND class boundaries."""
    cuts = sorted({c * W for c in range(NBK + 1)} | {mk_b1, mk_b2})
    subs = [(cuts[i], cuts[i + 1]) for i in range(len(cuts) - 1)]
    return subs


def _dve_chunks(mk_b1, mk_b2):
    """Per-tile bank chunk sets handled by the Vector engine (rest ACT).

    Boundary-containing chunks go to DVE (its per-call overhead is tiny,
    ACT pays ~600ns per extra call+accum-read).  DVE's dual-pass chunk
    costs ~4.9us vs ACT ~2.3us but starts later (waits for its bank
    pieces), so it gets 5 chunks to ACT's 13."""
    env = os.environ.get("SUPCON_DVE_CHUNKS")
    if env is not None:
        parts = env.split(";")
        return {t: {int(x) for x in parts[t].split(",") if x != ""}
                for t in range(NT)}
    b1, b2 = mk_b1 // W, mk_b2 // W
    d0 = {b1, b2}
    for cand in (7, 3, 0, 6, 1, 4):
        if len(d0) >= 3:
            break
        d0.add(cand)
    return {0: d0, 1: {b1, b2}}


def _emit_order(dvec):
    """Emission order of the 18 big chunks, chosen by a small pipeline
    model + hillclimb: 2 PSUM buffers rotate over allocations; a DVE
    chunk releases its buffer at pass-A end but pass A queues behind the
    previous pass B; piece arrivals follow the two-queue DMA plan."""
    arr_bb = 11.0
    arr_bk = [21.0, 17.4, 13.9, 25.3, 24.6, 18.2, 25.3, 18.2]

    def arrival(c):
        t, j = divmod(c, 9)
        return arr_bb if j == 0 else arr_bk[j - 1]

    def is_dve(c):
        t, j = divmod(c, 9)
        return j > 0 and (j - 1) in dvec[t]

    def sim(order, pe_t=1.0, act_t=2.3, dveA=2.4, dveB=2.5, start=7.0):
        pe = act = dve = start
        rel = [start, start]
        end = 0.0
        for i, c in enumerate(order):
            b = i % 2
            fd = max(pe, rel[b], arrival(c)) + pe_t
            pe = fd
            if is_dve(c):
                eA = max(dve, fd) + dveA
                dve = eA + dveB
                rel[b] = eA
                end = max(end, dve)
            else:
                e = max(act, fd) + act_t
                act = e
                rel[b] = e
                end = max(end, e)
        return end

    import random
    rng = random.Random(7)
    ids = list(range(18))
    best, bestm = None, None
    for restart in range(30):
        cur = rng.sample(ids, 18)
        curm = sim(cur)
        improved = True
        while improved:
            improved = False
            for a in range(18):
                for b in range(a + 1, 18):
                    o = cur[:]
                    o[a], o[b] = o[b], o[a]
                    m = sim(o)
                    if m < curm - 1e-9:
                        cur, curm = o, m
                        improved = True
        if bestm is None or curm < bestm:
            best, bestm = cur, curm
    return [divmod(c, 9) for c in best]


def _build(mk_b1, mk_b2):
    import ml_dtypes  # noqa: F401

    ops = _register_exp_ops()
    EXP_A = ops["EXP_POLY_A_ANT"]
    EXP_B = ops["EXP_SQ7_RED_ANT"]

    nc = bacc.Bacc()
    # anchT = [scaled anchors (APC) | unscaled anchors (APC) | g_c (C)]
    embT_d = nc.declare_dram_parameter("embT", [D, B], BF16, isOutput=False)
    anchT_d = nc.declare_dram_parameter("anchT", [D, 2 * APC + C], BF16, isOutput=False)
    bankT_d = nc.declare_dram_parameter("bankT", [D, M], BF16, isOutput=False)
    subs = _bank_subranges(mk_b1, mk_b2)
    NK = len(subs)
    dvec = _dve_chunks(mk_b1, mk_b2)
    # vecs: [invt | ninvt | invpc | coefv | cA | oneh | incl | kcol | eye]
    NV = NT * (5 + C + NK) + 1 + 128
    vecs_d = nc.declare_dram_parameter("vecs", [128, NV], F32, isOutput=False)
    oout_d = nc.declare_dram_parameter("oout", [128, 2 * NT], F32, isOutput=True)

    with tile.TileContext(nc) as tc:
        with (
            tc.tile_pool(name="big", bufs=1) as bigp,
            tc.tile_pool(name="sm", bufs=1) as smp,
            tc.tile_pool(name="ps", bufs=2, space="PSUM") as psp,
        ):
            anch_t = bigp.tile([D, 2 * APC + C], BF16, tag="anchT")
            vecs_t = smp.tile([128, NV], F32, tag="vecs")
            junkw_t = bigp.tile([128, 128], BF16, tag="junkw")
            junkx_t = bigp.tile([128, CH], BF16, tag="junkx")
            o = [0]

            def vslice(w):
                a = o[0]; o[0] += w
                return vecs_t[:, a:a + w]

            invt_t = vslice(NT)
            ninvt_t = vslice(NT)
            invpc_t = vslice(NT)
            coefv_t = vslice(NT)
            cA_t = vslice(NT)
            oneh_t = vslice(NT * C)
            incl_t = vslice(NT * NK)
            kcol_t = vslice(1)
            eye_t = vslice(128)

            emb_t = bigp.tile([D, B], BF16, tag="embT")
            # bank pieces live in DRAM permuted as BPERM so the scalar
            # queue's four pieces are contiguous pairs (merged DMAs keep
            # the scalar engine at 4 issue instructions -- the HWDGE queue
            # stalls the issuing engine beyond ~5 outstanding).
            bsingle = [bigp.tile([D, W], BF16, tag=f"bksl{i}", name=f"bksl{i}")
                       for i in range(4)]
            bmerge = [bigp.tile([D, 2 * W], BF16, tag=f"bkm{i}", name=f"bkm{i}")
                      for i in range(2)]

            def bank_view(j):
                s = BPERM.index(j)
                if s < 4:
                    return bsingle[s], 0
                m, half = divmod(s - 4, 2)
                return bmerge[m], half * W

            H = B // 2
            nc.sync.dma_start(out=vecs_t[:], in_=vecs_d[:])
            nc.scalar.dma_start(out=anch_t[:], in_=anchT_d[:])
            nc.sync.dma_start(out=emb_t[:, 0:H], in_=embT_d[:, 0:H])
            nc.scalar.dma_start(out=emb_t[:, H:B], in_=embT_d[:, H:B])
            # warm exp here: the ACT table load runs after only two issue
            # instructions, before the merged bank issues
            warm = smp.tile([128, 1], F32, tag="warm")
            nc.scalar.activation(warm[:], vecs_t[:, 0:1], AF.Exp)
            for i in range(4):
                nc.sync.dma_start(out=bsingle[i][:],
                                  in_=bankT_d[:, i * W:(i + 1) * W])
            for m in range(2):
                nc.scalar.dma_start(
                    out=bmerge[m][:],
                    in_=bankT_d[:, (4 + 2 * m) * W:(6 + 2 * m) * W])

            oout_t = smp.tile([128, 2 * NT], F32, tag="oout")
            scr_t = smp.tile([128, W], F32, tag="scrshared")
            wbuf_t = smp.tile([128, W], F32, tag="wbuf")
            dumb_t = smp.tile([128, W], F32, tag="dumb")
            sdiag = [smp.tile([128, 1], F32, tag=f"sdiag{t}", name=f"sdiag{t}") for t in range(NT)]
            selfe = [smp.tile([128, 1], F32, tag=f"selfe{t}", name=f"selfe{t}") for t in range(NT)]
            eyemul = smp.tile([128, 128], F32, tag="eyemul")
            bbsum = [smp.tile([128, 1], F32, tag=f"bbsum{t}", name=f"bbsum{t}") for t in range(NT)]
            raw3 = [smp.tile([128, C], F32, tag=f"raw3{t}", name=f"raw3{t}") for t in range(NT)]
            esum = [smp.tile([128, NK], F32, tag=f"esum{t}", name=f"esum{t}") for t in range(NT)]

            def anch(t):
                return anch_t[:, t * 128:(t + 1) * 128]

            def anchu(t):
                return anch_t[:, APC + t * 128:APC + (t + 1) * 128]

            # HAM warmup: PE activity from program start so the clock is at
            # 2.4GHz when the stream begins (junk operands, results unread;
            # memsets on gpsimd so the vector engine isn't delayed)
            nc.gpsimd.memset(junkw_t[:], 0.0)
            nc.gpsimd.memset(junkx_t[:], 0.0)
            warm_ps = psp.tile([128, W], F32, tag="chunk", name="warm_ps")
            for w in range(5):
                nc.tensor.matmul(
                    warm_ps[:, (w % 4) * CH:((w % 4) + 1) * CH],
                    junkw_t[:], junkx_t[:], start=True, stop=True,
                )

            # ---- prelude: self blocks (scaled x unscaled) + positives ----
            pre_ps = psp.tile([128, W], F32, tag="chunk", name="pre_ps")
            for t in range(NT):
                nc.tensor.matmul(
                    pre_ps[:, t * 128:(t + 1) * 128], anch(t), anchu(t),
                    start=True, stop=True,
                )
            for t in range(NT):
                nc.tensor.matmul(
                    pre_ps[:, 256 + t * C:256 + (t + 1) * C], anch(t),
                    anch_t[:, 2 * APC:2 * APC + C], start=True, stop=True,
                )
            for t in range(NT):
                nc.vector.tensor_mul(eyemul[:], pre_ps[:, t * 128:(t + 1) * 128], eye_t[:])
                nc.vector.reduce_sum(sdiag[t][:], eyemul[:], axis=AX.X)
                nc.vector.tensor_copy(out=raw3[t][:], in_=pre_ps[:, 256 + t * C:256 + (t + 1) * C])
                nc.scalar.activation(
                    selfe[t][:], sdiag[t][:], AF.Exp,
                    bias=ninvt_t[:, t:t + 1], scale=ACT_SCALE,
                )

            by_chunk = {}
            for k, (s, e) in enumerate(subs):
                by_chunk.setdefault(s // W, []).append((s, e, k))

            scrNK = [smp.tile([128, NK], F32, tag=f"scrNK{t}", name=f"scrNK{t}") for t in range(NT)]
            scrC = [smp.tile([128, C], F32, tag=f"scrC{t}", name=f"scrC{t}") for t in range(NT)]

            def epi_early(t):
                """olin = coefv*invt*(1 - pos): prelude-only deps."""
                own_r = smp.tile([128, 1], F32, tag=f"ownr{t}", name=f"ownr{t}")
                pos = smp.tile([128, 1], F32, tag=f"pos{t}", name=f"pos{t}")
                w1 = smp.tile([128, 1], F32, tag=f"w1{t}", name=f"w1{t}")
                p1 = smp.tile([128, 1], F32, tag=f"p1{t}", name=f"p1{t}")
                nc.vector.tensor_mul(scrC[t][:], raw3[t][:], oneh_t[:, t * C:(t + 1) * C])
                nc.vector.reduce_sum(own_r[:], scrC[t][:], axis=AX.X)
                nc.vector.scalar_tensor_tensor(
                    out=pos[:], in0=own_r[:], scalar=sdiag[t][:], in1=invpc_t[:, t:t + 1],
                    op0=ALU.subtract, op1=ALU.mult,
                )
                nc.vector.scalar_tensor_tensor(
                    out=w1[:], in0=pos[:], scalar=-1.0, in1=invt_t[:, t:t + 1],
                    op0=ALU.mult, op1=ALU.mult,
                )
                nc.vector.scalar_tensor_tensor(
                    out=oout_t[:, NT + t:NT + t + 1], in0=w1[:], scalar=invt_t[:, t:t + 1],
                    in1=coefv_t[:, t:t + 1], op0=ALU.add, op1=ALU.mult,
                )
                return p1

            p1s = {}

            def epilogue(t):
                nc.vector.tensor_mul(scrNK[t][:], esum[t][:], incl_t[:, t * NK:(t + 1) * NK])
                nc.vector.reduce_sum(oout_t[:, t:t + 1], scrNK[t][:], axis=AX.X)
                nc.vector.tensor_add(oout_t[:, t:t + 1], oout_t[:, t:t + 1], p1s[t][:])

            def emit_bb(t):
                ps = psp.tile([128, W], F32, tag="chunk", name="bb_ps")
                for q in range(W // CH):
                    nc.tensor.matmul(
                        ps[:, q * CH:(q + 1) * CH], anch(t),
                        emb_t[:, q * CH:(q + 1) * CH],
                        start=True, stop=True,
                    )
                nc.scalar.activation(
                    scr_t[:], ps[:], AF.Exp,
                    bias=ninvt_t[:, t:t + 1], scale=ACT_SCALE,
                    accum_out=bbsum[t][:],
                )

            def emit_bank_act(t, j):
                bt, off = bank_view(j)
                ps = psp.tile([128, W], F32, tag="chunk", name="bk_ps")
                for q in range(W // CH):
                    nc.tensor.matmul(
                        ps[:, q * CH:(q + 1) * CH], anch(t),
                        bt[:, off + q * CH:off + (q + 1) * CH],
                        start=True, stop=True,
                    )
                for (s, e, k) in by_chunk[j]:
                    a, b = s - j * W, e - j * W
                    nc.scalar.activation(
                        scr_t[:, a:b], ps[:, a:b], AF.Exp,
                        bias=ninvt_t[:, t:t + 1], scale=ACT_SCALE,
                        accum_out=esum[t][:, k:k + 1],
                    )

            dve_test = os.environ.get("SUPCON_DVE_TEST", "")

            def emit_bank_dve(t, j):
                bt, off = bank_view(j)
                ps = psp.tile([128, W], F32, tag="chunk", name="dk_ps")
                for q in range(W // CH):
                    nc.tensor.matmul(
                        ps[:, q * CH:(q + 1) * CH], anch(t),
                        bt[:, off + q * CH:off + (q + 1) * CH],
                        start=True, stop=True,
                    )
                if dve_test == "stock":
                    # stock custom-dve op in place of pass A: wrong math,
                    # tests whether ANY custom-dve runs on this device
                    nc.vector.reciprocal_approx_fast(out=wbuf_t[:], in_=ps[:])
                else:
                    nc.vector._custom_dve(
                        EXP_A, out=wbuf_t[:], in0=ps[:],
                        s0=cA_t[:, t:t + 1], s1=EXP_BPK, imm2=EXP_GPK,
                    )
                for (s, e, k) in by_chunk[j]:
                    a, b = s - j * W, e - j * W
                    if dve_test in ("stock", "a"):
                        nc.vector.memset(esum[t][:, k:k + 1], 1.0)
                    else:
                        nc.vector._custom_dve(
                            EXP_B, out=dumb_t[:, a:b], in0=wbuf_t[:, a:b],
                            s0=0.0, accum_out=esum[t][:, k:k + 1],
                        )

            def emit_bank(t, j):
                if j in dvec[t]:
                    emit_bank_dve(t, j)
                else:
                    emit_bank_act(t, j)

            for t in range(NT):
                p1s[t] = epi_early(t)
            remaining = {t: NBK + 1 for t in range(NT)}
            for (t, j) in _emit_order(dvec):
                if j == 0:
                    emit_bb(t)
                    nc.vector.tensor_sub(p1s[t][:], bbsum[t][:], selfe[t][:])
                else:
                    emit_bank(t, j - 1)
                remaining[t] -= 1
                if remaining[t] == 0:
                    epilogue(t)

            nc.sync.dma_start(out=oout_d[:], in_=oout_t[:])

    nc.compile()
    return nc


def _per_core_cols(vec, core):
    """[B] host vector -> [128, NT] tile for one core (col t, partition p)."""
    sl = vec[core * APC:(core + 1) * APC]
    return np.ascontiguousarray(sl.reshape(NT, 128).T).astype(np.float32)


def kernel(embeddings, labels, bank_embs, bank_labels, class_temps):
    global LAST_EXEC_TIME_NS
    import ml_dtypes

    emb = np.asarray(embeddings, dtype=np.float32)
    bank = np.asarray(bank_embs, dtype=np.float32)
    lab = np.asarray(labels).astype(np.int64).ravel()
    blab = np.asarray(bank_labels).astype(np.int64).ravel()
    ct = np.asarray(class_temps, dtype=np.float32).ravel()

    bord = np.argsort(lab, kind="stable")
    slab = lab[bord]
    mord = np.argsort(blab, kind="stable")
    cnt = np.bincount(lab, minlength=C)
    mcnt = np.bincount(blab, minlength=C)
    mk_b1, mk_b2 = int(mcnt[0]), int(mcnt[0] + mcnt[1])

    embT = np.ascontiguousarray(emb[bord].T).astype(ml_dtypes.bfloat16)  # [D, B]
    bankT0 = np.ascontiguousarray(bank[mord].T).astype(ml_dtypes.bfloat16)  # [D, M]
    bankT = np.ascontiguousarray(np.concatenate(
        [bankT0[:, j * W:(j + 1) * W] for j in BPERM], axis=1))

    temps = ct[slab]
    inv_t = (1.0 / temps).astype(np.float32)
    pos_cnt = cnt[slab] - 1
    # positives matmul is scaled by inv_t/128 (pre-scaled anchors)
    invpc = (128.0 / KCBRT / inv_t / np.maximum(pos_cnt, 1)).astype(np.float32)
    validf = (pos_cnt > 0).astype(np.float32)
    coefv = (BASE_TEMP / temps).astype(np.float32) * validf
    oneh = np.eye(C, dtype=np.float32)[slab]      # [B, 3]
    n_valid = int((pos_cnt > 0).sum())

    nc = _build(mk_b1, mk_b2)

    subs = _bank_subranges(mk_b1, mk_b2)
    NK = len(subs)
    sub_cls = np.array([0 if s < mk_b1 else (1 if s < mk_b2 else 2) for s, _ in subs])
    incl_full = (sub_cls[None, :] != slab[:, None]).astype(np.float32)  # [B, NK]
    eye128 = np.eye(128, dtype=np.float32)

    # per-class embedding-sum vectors for the positives matmul (unscaled)
    gT = np.stack([emb[bord][slab == c].sum(axis=0) for c in range(C)], axis=1)
    gT = np.ascontiguousarray(gT).astype(ml_dtypes.bfloat16)

    # DVE pass A per-anchor shift: C0 = -inv_t/128 - u0
    cA = (KCBRT * (-inv_t / 128.0 - EXP_U0)).astype(np.float32)
    kcol = np.full((128, 1), EXP_K, dtype=np.float32)

    in_maps = []
    for core in range(NCORES):
        asl = slice(core * APC, (core + 1) * APC)
        oh = oneh[asl].reshape(NT, 128, C).transpose(1, 0, 2).reshape(128, NT * C)
        ic = incl_full[asl].reshape(NT, 128, NK).transpose(1, 0, 2).reshape(128, NT * NK)
        vecs = np.concatenate([
            _per_core_cols(inv_t, core),
            _per_core_cols(-inv_t, core),
            _per_core_cols(invpc, core),
            _per_core_cols(coefv, core),
            _per_core_cols(cA, core),
            oh.astype(np.float32),
            ic.astype(np.float32),
            kcol,
            eye128,
        ], axis=1)
        # scaled anchors: columns * inv_t_i/128 (scale BEFORE bf16 cast)
        anch_sc = (emb[bord][asl] * (inv_t[asl, None] / 128.0 * KCBRT)).T.astype(ml_dtypes.bfloat16)
        anch_un = embT[:, asl]
        in_maps.append({
            "embT": embT,
            "anchT": np.ascontiguousarray(
                np.concatenate([anch_sc, anch_un, gT], axis=1)),
            "bankT": bankT,
            "vecs": np.ascontiguousarray(vecs),
        })

    trace = os.environ.get("SUPCON_TRACE", "0") == "1"
    if trace:
        trace = _install_trace_shim()
    res = run_bass_kernel_spmd(nc, in_maps, core_ids=list(range(NCORES)), trace=trace)
    LAST_EXEC_TIME_NS = res.exec_time_ns

    # loss_i = coef_i * log(den_i) + lin_i; host finishes logs + masked mean
    loss_sum = np.float64(0.0)
    for core in range(NCORES):
        oo = np.asarray(res.results[core]["oout"], dtype=np.float64)    # [128, 2*NT]
        den, lin = oo[:, :NT], oo[:, NT:]
        cf = _per_core_cols(coefv, core).astype(np.float64)
        loss_sum += (cf * np.log(den) + lin).sum()
    return np.float32(loss_sum / max(n_valid, 1))
